# revision 24
# baseline (speedup 1.0000x reference)
"""GCN encoder (2-layer PyG-style GCNConv) as a Bass/Tile kernel on 8 trn2 NeuronCores.

Strategy (graph/data parallel, per sharding hint):
  - Nodes are partitioned across the 8 cores (12544 padded positions each, in
    degree-balanced serpentine order); each core aggregates all edges whose
    destination lands in its shard.
  - The feature table is SHARDED: each core uploads only its own 12544-row
    slice (bf16, dinv[src]-prescaled); the full table is assembled on-device
    with 4 quarter-chunk AllGathers (same flow as the layer-1 hidden table).
  - Both layers use identical position-based indexing, so the gather-index
    stream and the dst-selector stream are shared between layers and stay
    SBUF-resident.
  - Aggregation per 128-node dst block: for each 128-edge tile, gather the
    source rows with dma_gather (bf16), then accumulate indicator-weighted
    messages into PSUM via PE matmuls: aggT[f,d] += msg[e,f]^T @ ind[e,d].
  - Indicators are built 4 slots at a time: a K=4 PE matmul broadcasts the
    4 slots' dst-selector rows into a [128, 512] PSUM tile, and ONE wide DVE
    is_equal against a tiled iota constant produces all 4 indicator blocks
    (4x fewer DVE instructions than per-slot tensor_scalar builds).
  - GCN normalization deg^-1/2 A_hat deg^-1/2: table rows pre-scaled by
    dinv[src]; dinv[dst] applied per block after the dense transform.
"""

import sys

sys.path.insert(0, "/opt/trn_rl_repo")

import numpy as np

import concourse.bass as bass
import concourse.bacc as bacc
import concourse.mybir as mybir
from concourse import tile, library_config

BF16 = mybir.dt.bfloat16
F32 = mybir.dt.float32
I16 = mybir.dt.int16
BF16_NP = mybir.dt.np(BF16)

DIN, DH, DOUT = 128, 128, 64
SB = 4  # indicator slots built per SELB matmul / wide is_equal


def make_cfg(n_nodes, n_edges, n_cores=8, bpc=98, bpg=7, q_blocks=(25, 25, 24, 24),
             gcap=896, n_queues=2):
    cfg = {}
    cfg["N"] = n_nodes
    cfg["E"] = n_edges
    cfg["GCAP"] = gcap          # max indices per dma_gather instruction
    cfg["NQ"] = n_queues        # SWDGE queues to spread gathers over
    cfg["NCORES"] = n_cores
    cfg["BPC"] = bpc                      # dst blocks (of 128 nodes) per core
    cfg["BPG"] = bpg                      # blocks per gather group
    assert bpc % bpg == 0
    cfg["NG"] = bpc // bpg                # gather groups per core
    cfg["SHARD"] = bpc * 128              # padded nodes per core
    cfg["NP"] = n_cores * cfg["SHARD"]    # padded total nodes
    assert cfg["NP"] >= n_nodes
    assert sum(q_blocks) == bpc and len(q_blocks) == 4
    cfg["QB"] = list(q_blocks)            # blocks per quarter (collective chunks)
    cfg["QSTART"] = np.concatenate([[0], np.cumsum(q_blocks)])  # block ids
    cfg["QN"] = [q * 128 for q in q_blocks]   # nodes per quarter per rank
    for q in q_blocks:
        assert q * 128 * n_cores <= 32767
    return cfg


def _block_quarter(cfg, blk):
    """quarter id for a block index (vectorized)."""
    return np.searchsorted(cfg["QSTART"][1:], blk, side="right")


def make_layout(cfg, L):
    """Static slot/position layout from the padded per-(block, chunk) length
    table L [BPC, 4] (multiples of 128, identical across cores).

    Global ordering: group-major, then chunk, then block within group."""
    BPC, BPG, NG = cfg["BPC"], cfg["BPG"], cfg["NG"]
    gpos = np.zeros((BPC, 4), np.int64)      # global position base of run (b, c)
    run_len = np.zeros((NG, 4), np.int64)    # positions per (g, c) gather
    grp_base = np.zeros(NG + 1, np.int64)    # global position base of group g
    p = 0
    for g in range(NG):
        grp_base[g] = p
        for c in range(4):
            for b in range(g * BPG, (g + 1) * BPG):
                gpos[b, c] = p
                p += L[b, c]
            run_len[g, c] = p - (gpos[g * BPG, c])
    grp_base[NG] = p

    # per-block slot lists in consumption order + 4-aligned SELT row layout
    blk_slots = []
    blk_selt0 = []
    selt_row = np.full(p // 128, -1, np.int64)
    r = 0
    for b in range(BPC):
        slots = []
        for c in range(4):
            s0 = int(gpos[b, c])
            for s in range(s0 // 128, (s0 + int(L[b, c])) // 128):
                slots.append(s)
        blk_slots.append(slots)
        blk_selt0.append(r)
        for s in slots:
            selt_row[s] = r
            r += 1
        r = (r + SB - 1) // SB * SB
    return {
        "gpos": gpos,
        "run_len": run_len,
        "grp_base": grp_base,
        "total_pos": p,
        "total_slots": p // 128,
        "blk_slots": blk_slots,
        "blk_selt0": blk_selt0,
        "selt_row": selt_row,
        "n_selt": r,
    }


def preprocess(cfg, x, edge_index, W1, b1, W2, b2):
    """Host-side sharding: bucket/sort edges, build the shared per-core gather
    index + dst-selector streams, degree normalization, bf16 tables."""
    N, NP, NC = cfg["N"], cfg["NP"], cfg["NCORES"]
    SHARD, BPC, BPG = cfg["SHARD"], cfg["BPC"], cfg["BPG"]

    x = np.asarray(x, np.float32)
    edge_index = np.asarray(edge_index)
    W1 = np.asarray(W1, np.float32)
    b1 = np.asarray(b1, np.float32)
    W2 = np.asarray(W2, np.float32)
    b2 = np.asarray(b2, np.float32)

    # self-loops are NOT in the gather stream: they are added per block on
    # device via an identity matmul from the locally-kept (prescaled) tables
    src = edge_index[0].astype(np.int64)
    dst = edge_index[1].astype(np.int64)

    deg = np.bincount(dst, minlength=NP).astype(np.float32)
    deg[:N] += 1.0                      # A_hat = A + I
    dinv = np.zeros(NP, np.float32)
    nz = deg > 0
    dinv[nz] = 1.0 / np.sqrt(deg[nz])

    # degree-balanced node -> (core, block, slot) packing: serpentine deal of
    # nodes sorted by in-degree so every 128-node block has ~equal edge count
    NB = NP // 128
    order = np.argsort(-deg[:N], kind="stable")
    ids = np.concatenate([order, np.full(NP - N, -1, np.int64)])
    rounds = ids.reshape(128, NB).copy()
    rounds[1::2] = rounds[1::2, ::-1]
    posmat = (np.arange(NB)[None, :] * 128 + np.arange(128)[:, None])
    node_pos = np.zeros(N, np.int64)
    m = rounds >= 0
    node_pos[rounds[m]] = posmat[m]

    # position-ordered dinv[src]-prescaled bf16 table (sharded per core)
    xs = np.zeros((NP, DIN), np.float32)
    xs[node_pos] = x * dinv[:N, None]
    xt = xs.astype(BF16_NP)

    p_dst = node_pos[dst]
    core = (p_dst // SHARD).astype(np.int32)
    blk = ((p_dst % SHARD) // 128).astype(np.int32)   # block within core
    dloc = (p_dst % 128).astype(np.int32)
    grp = blk // BPG

    # shared (both layers) position-quarter chunking of the source index
    p_src = node_pos[src]
    s_rank = p_src // SHARD
    s_loc = p_src % SHARD
    s_blk = (s_loc // 128).astype(np.int32)
    cl = _block_quarter(cfg, s_blk).astype(np.int32)
    qn = np.asarray(cfg["QN"], np.int64)
    qstart_nodes = cfg["QSTART"][:4] * 128
    ival = (s_rank * qn[cl] + (s_loc - qstart_nodes[cl])).astype(np.int16)

    # per-core per-(block, chunk) counts -> shared padded length table
    key = (core.astype(np.int64) * BPC + blk) * 4 + cl
    cnt = np.bincount(key, minlength=NC * BPC * 4).reshape(NC, BPC, 4)
    mx = cnt.max(axis=0)
    L = ((mx + 127) // 128) * 128
    empty = L.sum(axis=1) == 0
    L[empty, 0] = 128          # every block needs >=1 tile so PSUM gets written
    lay = make_layout(cfg, L)

    # stable sort: (core, group, chunk, block, src)
    sorder = np.lexsort((src, blk, cl, grp, core))
    ekey = key[sorder]
    change = np.r_[True, ekey[1:] != ekey[:-1]]
    starts = np.flatnonzero(change)
    runid = np.cumsum(change) - 1
    within = np.arange(len(ekey)) - starts[runid]
    gp = lay["gpos"]
    pos = gp[blk[sorder], cl[sorder]] + within
    core_o = core[sorder]

    total = lay["total_pos"]
    nslot = lay["total_slots"]
    gp_flat = gp.reshape(-1)
    L_flat = L.reshape(-1)
    # SELT row r -> selt4[r % SB, (r // SB)*128 + lane]
    srow = lay["selt_row"]                   # [nslot]
    NBAT = lay["n_selt"] // SB
    lanes = np.arange(128)

    in_maps = [dict() for _ in range(NC)]
    for r in range(NC):
        mrk = core_o == r
        iarr = np.zeros(total, np.int16)
        sarr = np.full(total, -1.0, np.float32)
        iarr[pos[mrk]] = ival[sorder][mrk]
        sarr[pos[mrk]] = dloc[sorder][mrk].astype(np.float32)
        # forward-fill pad positions with the run's first real index so
        # pad gathers hit nearby/cached table rows
        cnt_r = cnt[r].reshape(-1)
        has = cnt_r > 0
        firsts = np.zeros(len(L_flat), np.int16)
        firsts[has] = iarr[gp_flat[has]]
        ordr = np.argsort(gp_flat, kind="stable")
        run_of_pos = np.repeat(ordr, L_flat[ordr])
        off_of_pos = np.arange(total) - np.repeat(gp_flat[ordr], L_flat[ordr])
        padmask = off_of_pos >= cnt_r[run_of_pos]
        iarr[padmask] = firsts[run_of_pos[padmask]]
        # [16, total/16] wrapped layout; replicated to 128 partitions on-device
        in_maps[r]["idx"] = np.ascontiguousarray(iarr.reshape(-1, 16).T)

        selt = np.full((SB, NBAT * 128), -1.0, np.float32)
        part = srow % SB
        colb = (srow // SB) * 128
        selt[part[:, None], colb[:, None] + lanes[None, :]] = \
            sarr.reshape(nslot, 128)
        in_maps[r]["selt"] = np.ascontiguousarray(selt.astype(BF16_NP))

    rhs4 = np.zeros((SB, SB * 128), np.float32)
    for k in range(SB):
        rhs4[k, k * 128:(k + 1) * 128] = 1.0
    w1s = W1.astype(BF16_NP)
    w2s = W2.astype(BF16_NP)
    b1b = np.tile(b1, (128, 1)).astype(np.float32)
    b2b = np.tile(b2, (128, 1)).astype(np.float32)

    dinv_by_pos = np.zeros(NP, np.float32)
    dinv_by_pos[node_pos] = dinv[:N]
    for r in range(NC):
        sh = dinv_by_pos[r * SHARD:(r + 1) * SHARD]
        in_maps[r]["dinv"] = np.ascontiguousarray(sh.reshape(BPC, 128).T)
        in_maps[r]["xt"] = np.ascontiguousarray(xt[r * SHARD:(r + 1) * SHARD])
        in_maps[r]["w1s"] = w1s
        in_maps[r]["w2s"] = w2s
        in_maps[r]["b1b"] = b1b
        in_maps[r]["b2b"] = b2b
        in_maps[r]["idm"] = np.eye(128, dtype=BF16_NP)
        in_maps[r]["rhs4"] = rhs4.astype(BF16_NP)

    return in_maps, L, lay, node_pos


def build_nc(cfg, L, lay, debug=False, sim_single=False):
    NC, BPC, BPG, NG = cfg["NCORES"], cfg["BPC"], cfg["BPG"], cfg["NG"]
    SHARD = cfg["SHARD"]
    QB, QN, QSTART = cfg["QB"], cfg["QN"], cfg["QSTART"]
    GCAP, NQ = cfg["GCAP"], cfg["NQ"]

    nc = bacc.Bacc("TRN2", target_bir_lowering=False, debug=debug,
                   num_devices=1 if sim_single else NC,
                   num_swdge_queues=NQ)

    total_pos = lay["total_pos"]
    NBAT = lay["n_selt"] // SB
    # per-group SELT batch ranges (block selt rows are SB-aligned)
    bs0 = lay["blk_selt0"] + [lay["n_selt"]]
    grp_bat0 = [bs0[g * BPG] // SB for g in range(NG + 1)]
    max_grp_bat = max(grp_bat0[g + 1] - grp_bat0[g] for g in range(NG))

    t_xt = nc.dram_tensor("xt", [SHARD, DIN], BF16, kind="ExternalInput")
    t_w1 = nc.dram_tensor("w1s", [DIN, DH], BF16, kind="ExternalInput")
    t_w2 = nc.dram_tensor("w2s", [DH, DOUT], BF16, kind="ExternalInput")
    t_b1 = nc.dram_tensor("b1b", [128, DH], F32, kind="ExternalInput")
    t_b2 = nc.dram_tensor("b2b", [128, DOUT], F32, kind="ExternalInput")
    t_idm = nc.dram_tensor("idm", [128, 128], BF16, kind="ExternalInput")
    t_rhs4 = nc.dram_tensor("rhs4", [SB, SB * 128], BF16, kind="ExternalInput")
    t_dinv = nc.dram_tensor("dinv", [128, BPC], F32, kind="ExternalInput")
    t_idx = nc.dram_tensor("idx", [16, total_pos // 16], I16, kind="ExternalInput")
    t_selt = nc.dram_tensor("selt", [SB, NBAT * 128], BF16, kind="ExternalInput")
    t_out = nc.dram_tensor("out", [SHARD, DOUT], BF16, kind="ExternalOutput")

    max_grp_pos = max(
        int(lay["grp_base"][g + 1] - lay["grp_base"][g]) for g in range(NG)
    )

    with tile.TileContext(nc) as tc:
        with (
            tc.tile_pool(name="const", bufs=1) as constp,
            tc.tile_pool(name="dram", bufs=1, space="DRAM") as dramp,
            tc.tile_pool(name="msg", bufs=2) as msgp,
            tc.tile_pool(name="selg", bufs=2) as stagp,
            tc.tile_pool(name="ind", bufs=10) as indp,
            tc.tile_pool(name="selb", bufs=3, space="PSUM") as selbp,
            tc.tile_pool(name="aggps", bufs=2, space="PSUM") as aggpsp,
            tc.tile_pool(name="xfps", bufs=2, space="PSUM") as xfpsp,
            tc.tile_pool(name="post", bufs=4) as postp,
        ):
            nc.gpsimd.load_library(library_config.mlp)

            W1 = constp.tile([DIN, DH], BF16)
            nc.sync.dma_start(W1[:], t_w1[:, :])
            W2 = constp.tile([DH, DOUT], BF16)
            nc.sync.dma_start(W2[:], t_w2[:, :])
            B1 = constp.tile([128, DH], F32)
            nc.sync.dma_start(B1[:], t_b1[:, :])
            B2 = constp.tile([128, DOUT], F32)
            nc.sync.dma_start(B2[:], t_b2[:, :])
            ID = constp.tile([128, 128], BF16)
            nc.sync.dma_start(ID[:], t_idm[:, :])
            IOTA4I = constp.tile([128, SB * 128], mybir.dt.int32)
            nc.gpsimd.iota(IOTA4I[:], [[0, SB], [1, 128]], base=0,
                           channel_multiplier=0)
            IOTA4 = constp.tile([128, SB * 128], F32)
            nc.vector.tensor_copy(IOTA4[:], IOTA4I[:])
            RHS4 = constp.tile([SB, SB * 128], BF16)
            nc.sync.dma_start(RHS4[:], t_rhs4[:, :])
            DINV = constp.tile([128, BPC], F32)
            nc.sync.dma_start(DINV[:], t_dinv[:, :])
            IDX = constp.tile([128, total_pos // 16], I16)
            for k in range(8):   # replicate the 16-row idx stream on-device
                nc.sync.dma_start(IDX[k * 16:(k + 1) * 16, :], t_idx[:, :])
            # local shard tables kept in SBUF for the per-block self-loop
            # identity matmuls (dloc on partitions, block-major columns)
            XTKEEP = constp.tile([128, BPC * DIN], BF16)
            for b in range(BPC):
                nc.sync.dma_start(XTKEEP[:, b * DIN:(b + 1) * DIN],
                                  t_xt[b * 128:(b + 1) * 128, :])
            H1KEEP = constp.tile([128, BPC * DH], BF16)

            xt_mine = [dramp.tile([QN[q], DIN], BF16, name=f"xtmine{q}")
                       for q in range(4)]
            xt_tab = [dramp.tile([QN[q] * NC, DIN], BF16, addr_space="Shared",
                                 name=f"xttab{q}") for q in range(4)]
            h1_mine = [dramp.tile([QN[q], DH], BF16, name=f"h1mine{q}")
                       for q in range(4)]
            h1_tab = [dramp.tile([QN[q] * NC, DH], BF16, addr_space="Shared",
                                 name=f"h1tab{q}") for q in range(4)]

            # assemble the full source table on-device from the 1/8 shard
            # (collectives can't read IO tensors: bounce through a DRAM tile)
            for q in range(4):
                r0 = int(QSTART[q]) * 128
                r1 = int(QSTART[q + 1]) * 128
                nc.sync.dma_start(xt_mine[q][:, :], t_xt[r0:r1, :])
                if sim_single:
                    nc.sync.dma_start(xt_tab[q][:QN[q], :], xt_mine[q][:, :])
                else:
                    nc.gpsimd.collective_compute(
                        "AllGather",
                        mybir.AluOpType.bypass,
                        replica_groups=[list(range(NC))],
                        ins=[xt_mine[q].opt()],
                        outs=[xt_tab[q].opt()],
                    )

            gpos = lay["gpos"]
            grp_base = lay["grp_base"]
            blk_slots = lay["blk_slots"]
            blk_selt0 = lay["blk_selt0"]

            def do_layer(layer):
                tabs = xt_tab if layer == 0 else h1_tab
                for g in range(NG):
                    p0 = int(grp_base[g])
                    p1 = int(grp_base[g + 1])
                    if p1 == p0:
                        continue
                    msg = msgp.tile([128, max_grp_pos // 128, DH], BF16, tag="msg")
                    gb0 = grp_bat0[g]
                    ngb = grp_bat0[g + 1] - gb0
                    SELG = stagp.tile([SB, max_grp_bat * 128], BF16, tag="selg")
                    nc.sync.dma_start(SELG[:, :ngb * 128],
                                      t_selt[:, gb0 * 128:(gb0 + ngb) * 128])
                    gq = 0
                    for c in range(4):
                        nidx = int(lay["run_len"][g, c])
                        if nidx == 0:
                            continue
                        rp0 = int(gpos[g * BPG, c])  # global pos of run start
                        for s0 in range(0, nidx, GCAP):
                            n = min(GCAP, nidx - s0)
                            a0 = rp0 - p0 + s0       # pos offset in group buf
                            nc.gpsimd.dma_gather(
                                out_ap=msg[:, a0 // 128: (a0 + n) // 128, :],
                                in_ap=tabs[c][:, :],
                                idxs_ap=IDX[:, (rp0 + s0) // 16: (rp0 + s0 + n) // 16],
                                num_idxs=n,
                                num_idxs_reg=n,
                                elem_size=DH,
                                queue_num=gq % NQ,
                            )
                            gq += 1
                    for b in range(g * BPG, (g + 1) * BPG):
                        slots = blk_slots[b]
                        assert slots
                        nbat = (len(slots) + SB - 1) // SB
                        inds = []
                        for t in range(nbat):
                            bt = blk_selt0[b] // SB + t - gb0  # batch in group
                            selb = selbp.tile([128, SB * 128], F32, tag="selb")
                            nc.tensor.matmul(
                                selb[:],
                                lhsT=SELG[:, bt * 128:(bt + 1) * 128],
                                rhs=RHS4[:],
                                start=True, stop=True,
                            )
                            ind = indp.tile([128, SB * 128], BF16, tag="ind")
                            nc.vector.tensor_tensor(
                                ind[:], selb[:], IOTA4[:],
                                mybir.AluOpType.is_equal,
                            )
                            inds.append(ind)
                        psA = aggpsp.tile([DH, 128], F32, tag="aggps")
                        # self-loop: prescaled local row d contributes to col d
                        keep = XTKEEP if layer == 0 else H1KEEP
                        dk = DIN if layer == 0 else DH
                        nc.tensor.matmul(
                            psA[:], lhsT=keep[:, b * dk:(b + 1) * dk], rhs=ID[:],
                            start=True, stop=False,
                        )
                        for k, s in enumerate(slots):
                            nc.tensor.matmul(
                                psA[:],
                                lhsT=msg[:, s - p0 // 128, :],
                                rhs=inds[k // SB][:, (k % SB) * 128:(k % SB + 1) * 128],
                                start=False, stop=(k == len(slots) - 1),
                            )
                        aggs = postp.tile([DH, 128], BF16, tag="aggs")
                        nc.vector.tensor_copy(aggs[:], psA[:])
                        dcol = DINV[:, b: b + 1]
                        if layer == 0:
                            psH = xfpsp.tile([128, DH], F32, tag="xfps")
                            nc.tensor.matmul(psH[:], lhsT=aggs[:], rhs=W1[:],
                                             start=True, stop=True)
                            tA = postp.tile([128, DH], F32, tag="tA")
                            nc.any.tensor_scalar(tA[:], psH[:], dcol, None,
                                                 mybir.AluOpType.mult)
                            tB = postp.tile([128, DH], F32, tag="tB")
                            nc.any.tensor_tensor(tB[:], tA[:], B1[:],
                                                 mybir.AluOpType.add)
                            # relu(x)*d == relu(x*d) for d>=0: fold the layer-2
                            # dinv[src] table prescale into the activation;
                            # write straight into the kept table for the
                            # layer-2 self-loop matmul
                            h1s = H1KEEP[:, b * DH:(b + 1) * DH]
                            nc.scalar.activation(
                                h1s, tB[:],
                                mybir.ActivationFunctionType.Relu, scale=dcol,
                            )
                            q = int(_block_quarter(cfg, b))
                            r0 = (b - int(QSTART[q])) * 128
                            nc.sync.dma_start(h1_mine[q][r0:r0 + 128, :], h1s)
                            if b == int(QSTART[q + 1]) - 1:
                                if sim_single:
                                    nc.sync.dma_start(
                                        h1_tab[q][:QN[q], :], h1_mine[q][:, :])
                                else:
                                    nc.gpsimd.collective_compute(
                                        "AllGather",
                                        mybir.AluOpType.bypass,
                                        replica_groups=[list(range(NC))],
                                        ins=[h1_mine[q].opt()],
                                        outs=[h1_tab[q].opt()],
                                    )
                        else:
                            psO = xfpsp.tile([128, DOUT], F32, tag="xfps")
                            nc.tensor.matmul(psO[:], lhsT=aggs[:], rhs=W2[:],
                                             start=True, stop=True)
                            tA = postp.tile([128, DOUT], F32, tag="tA")
                            nc.any.tensor_scalar(tA[:], psO[:], dcol, None,
                                                 mybir.AluOpType.mult)
                            ot = postp.tile([128, DOUT], BF16, tag="ot")
                            nc.any.tensor_tensor(ot[:], tA[:], B2[:],
                                                 mybir.AluOpType.add)
                            nc.sync.dma_start(t_out[b * 128:(b + 1) * 128, :], ot[:])

            do_layer(0)
            do_layer(1)

    nc.compile()
    return nc


def kernel(x, edge_index, W1, b1, W2, b2):
    cfg = make_cfg(100000, 1600000)
    in_maps, L, lay, node_pos = preprocess(cfg, x, edge_index, W1, b1, W2, b2)
    nc = build_nc(cfg, L, lay, debug=False)
    from concourse import bass_utils
    res = bass_utils.run_bass_kernel_spmd(
        nc, in_maps, core_ids=list(range(cfg["NCORES"]))
    )
    out = np.concatenate([res.results[r]["out"] for r in range(cfg["NCORES"])],
                         axis=0)
    return np.ascontiguousarray(out[node_pos]).astype(np.float32)


# revision 34
# speedup vs baseline: 1.3189x; 1.3189x over previous
"""GCN encoder (2-layer PyG-style GCNConv) as a Bass/Tile kernel on 8 trn2 NeuronCores.

Strategy (graph/data parallel, per sharding hint):
  - Nodes are partitioned across the 8 cores (12544 padded positions each, in
    degree-balanced serpentine order); each core aggregates all edges whose
    destination lands in its shard.
  - The feature table is SHARDED: each core uploads only its own 12544-row
    slice (bf16, dinv[src]-prescaled); the full table is assembled on-device
    with 4 quarter-chunk AllGathers (same flow as the layer-1 hidden table).
  - Both layers use identical position-based indexing, so the gather-index
    stream and the dst-selector stream are shared between layers and stay
    SBUF-resident.
  - Aggregation per 128-node dst block: for each 128-edge tile, gather the
    source rows with dma_gather (bf16), then accumulate indicator-weighted
    messages into PSUM via PE matmuls: aggT[f,d] += msg[e,f]^T @ ind[e,d].
  - Indicators are built 4 slots at a time: a K=4 PE matmul broadcasts the
    4 slots' dst-selector rows into a [128, 512] PSUM tile, and ONE wide DVE
    is_equal against a tiled iota constant produces all 4 indicator blocks
    (4x fewer DVE instructions than per-slot tensor_scalar builds).
  - GCN normalization deg^-1/2 A_hat deg^-1/2: table rows pre-scaled by
    dinv[src]; dinv[dst] applied per block after the dense transform.
"""

import sys

sys.path.insert(0, "/opt/trn_rl_repo")

import numpy as np

import concourse.bass as bass
import concourse.bacc as bacc
import concourse.mybir as mybir
from concourse import tile, library_config

BF16 = mybir.dt.bfloat16
F32 = mybir.dt.float32
I16 = mybir.dt.int16
BF16_NP = mybir.dt.np(BF16)

DIN, DH, DOUT = 128, 128, 64
SB = 4  # indicator slots built per SELB matmul / wide is_equal


def make_cfg(n_nodes, n_edges, n_cores=8, bpc=98, bpg=7, q_blocks=(25, 25, 24, 24),
             gcap=896, n_queues=2, strip=True):
    cfg = {}
    # strip=True: run-tail pads are -1 so the Q7 ucode skips them (HW).
    # CoreSim NaN-poisons skipped lanes, so sim verification uses strip=False
    # (forward-filled pads; identical layout and instruction stream).
    cfg["STRIP"] = strip
    cfg["N"] = n_nodes
    cfg["E"] = n_edges
    cfg["GCAP"] = gcap          # max indices per dma_gather instruction
    cfg["NQ"] = n_queues        # SWDGE queues to spread gathers over
    cfg["NCORES"] = n_cores
    cfg["BPC"] = bpc                      # dst blocks (of 128 nodes) per core
    cfg["BPG"] = bpg                      # blocks per gather group
    assert bpc % bpg == 0
    cfg["NG"] = bpc // bpg                # gather groups per core
    cfg["SHARD"] = bpc * 128              # padded nodes per core
    cfg["NP"] = n_cores * cfg["SHARD"]    # padded total nodes
    assert cfg["NP"] >= n_nodes
    assert sum(q_blocks) == bpc and len(q_blocks) == 4
    cfg["QB"] = list(q_blocks)            # blocks per quarter (collective chunks)
    cfg["QSTART"] = np.concatenate([[0], np.cumsum(q_blocks)])  # block ids
    cfg["QN"] = [q * 128 for q in q_blocks]   # nodes per quarter per rank
    for q in q_blocks:
        assert q * 128 * n_cores <= 32767
    return cfg


def _block_quarter(cfg, blk):
    """quarter id for a block index (vectorized)."""
    return np.searchsorted(cfg["QSTART"][1:], blk, side="right")


def make_layout(cfg, L):
    """Static slot/position layout from the per-(block, chunk) length table
    L [BPC, 4] (max real count over cores, identical across cores).

    Runs are per (group, chunk) and quantized to 128 at RUN level only:
    blocks within a run abut directly, so 128-edge slots at block boundaries
    are shared by adjacent blocks (each processes the slot with a selector
    row that masks the other block's lanes)."""
    BPC, BPG, NG = cfg["BPC"], cfg["BPG"], cfg["NG"]
    gpos = np.zeros((BPC, 4), np.int64)      # global position base of (b, c)
    run_len = np.zeros((NG, 4), np.int64)    # padded positions per (g, c) run
    grp_base = np.zeros(NG + 1, np.int64)    # global position base of group g
    p = 0
    for g in range(NG):
        grp_base[g] = p
        for c in range(4):
            for b in range(g * BPG, (g + 1) * BPG):
                gpos[b, c] = p
                p += int(L[b, c])
            rl = p - int(gpos[g * BPG, c])
            rl_pad = (rl + 127) // 128 * 128
            run_len[g, c] = rl_pad
            p = int(gpos[g * BPG, c]) + rl_pad
    grp_base[NG] = p

    # per-block slot lists (incl. shared boundary slots) + 4-aligned SELT rows
    blk_slots = []
    blk_selt0 = []
    row_blk, row_slot = [], []
    r = 0
    for b in range(BPC):
        slots = []
        for c in range(4):
            s0 = int(gpos[b, c])
            n = int(L[b, c])
            if n == 0:
                continue
            for s in range(s0 // 128, (s0 + n - 1) // 128 + 1):
                slots.append(s)
        blk_slots.append(slots)
        blk_selt0.append(r)
        for s in slots:
            row_blk.append(b)
            row_slot.append(s)
            r += 1
        while r % SB:
            row_blk.append(-1)
            row_slot.append(0)
            r += 1
    return {
        "gpos": gpos,
        "run_len": run_len,
        "grp_base": grp_base,
        "total_pos": p,
        "total_slots": p // 128,
        "blk_slots": blk_slots,
        "blk_selt0": blk_selt0,
        "row_blk": np.asarray(row_blk, np.int64),
        "row_slot": np.asarray(row_slot, np.int64),
        "n_selt": r,
    }


def preprocess(cfg, x, edge_index, W1, b1, W2, b2):
    """Host-side sharding: bucket/sort edges, build the shared per-core gather
    index + dst-selector streams, degree normalization, bf16 tables."""
    N, NP, NC = cfg["N"], cfg["NP"], cfg["NCORES"]
    SHARD, BPC, BPG = cfg["SHARD"], cfg["BPC"], cfg["BPG"]

    x = np.asarray(x, np.float32)
    edge_index = np.asarray(edge_index)
    W1 = np.asarray(W1, np.float32)
    b1 = np.asarray(b1, np.float32)
    W2 = np.asarray(W2, np.float32)
    b2 = np.asarray(b2, np.float32)

    # self-loops are NOT in the gather stream: they are added per block on
    # device via an identity matmul from the locally-kept (prescaled) tables
    src = edge_index[0].astype(np.int64)
    dst = edge_index[1].astype(np.int64)

    deg = np.bincount(dst, minlength=NP).astype(np.float32)
    deg[:N] += 1.0                      # A_hat = A + I
    dinv = np.zeros(NP, np.float32)
    nz = deg > 0
    dinv[nz] = 1.0 / np.sqrt(deg[nz])

    # degree-balanced node -> (core, block, slot) packing: serpentine deal of
    # nodes sorted by in-degree so every 128-node block has ~equal edge count
    NB = NP // 128
    order = np.argsort(-deg[:N], kind="stable")
    ids = np.concatenate([order, np.full(NP - N, -1, np.int64)])
    rounds = ids.reshape(128, NB).copy()
    rounds[1::2] = rounds[1::2, ::-1]
    posmat = (np.arange(NB)[None, :] * 128 + np.arange(128)[:, None])
    node_pos = np.zeros(N, np.int64)
    m = rounds >= 0
    node_pos[rounds[m]] = posmat[m]

    # position-ordered dinv[src]-prescaled bf16 table (sharded per core)
    xs = np.zeros((NP, DIN), np.float32)
    xs[node_pos] = x * dinv[:N, None]
    xt = xs.astype(BF16_NP)

    p_dst = node_pos[dst]
    core = (p_dst // SHARD).astype(np.int32)
    blk = ((p_dst % SHARD) // 128).astype(np.int32)   # block within core
    dloc = (p_dst % 128).astype(np.int32)
    grp = blk // BPG

    # shared (both layers) position-quarter chunking of the source index
    p_src = node_pos[src]
    s_rank = p_src // SHARD
    s_loc = p_src % SHARD
    s_blk = (s_loc // 128).astype(np.int32)
    cl = _block_quarter(cfg, s_blk).astype(np.int32)
    qn = np.asarray(cfg["QN"], np.int64)
    qstart_nodes = cfg["QSTART"][:4] * 128
    ival = (s_rank * qn[cl] + (s_loc - qstart_nodes[cl])).astype(np.int16)

    # per-core per-(block, chunk) counts -> shared max-count length table
    key = (core.astype(np.int64) * BPC + blk) * 4 + cl
    cnt = np.bincount(key, minlength=NC * BPC * 4).reshape(NC, BPC, 4)
    L = cnt.max(axis=0)
    lay = make_layout(cfg, L)

    # stable sort: (core, group, chunk, block, src)
    sorder = np.lexsort((src, blk, cl, grp, core))
    ekey = key[sorder]
    change = np.r_[True, ekey[1:] != ekey[:-1]]
    starts = np.flatnonzero(change)
    runid = np.cumsum(change) - 1
    within = np.arange(len(ekey)) - starts[runid]
    gp = lay["gpos"]
    pos = gp[blk[sorder], cl[sorder]] + within
    core_o = core[sorder]

    total = lay["total_pos"]
    nslot = lay["total_slots"]
    gp_flat = gp.reshape(-1)
    L_flat = L.reshape(-1)
    NBAT = lay["n_selt"] // SB
    lanes = np.arange(128)

    # map every position to its owning (block, chunk) run; positions beyond
    # the runs' real extents (run-tail 128-rounding) have owner -1
    ordr = np.argsort(gp_flat, kind="stable")
    gp_sorted = gp_flat[ordr]
    L_sorted = L_flat[ordr]
    allpos = np.arange(total)
    prun = np.searchsorted(gp_sorted, allpos, side="right") - 1
    off_of_pos = allpos - gp_sorted[prun]
    covered = off_of_pos < L_sorted[prun]
    blk_of_pos = np.where(covered, ordr[prun] // 4, -1)

    row_blk, row_slot = lay["row_blk"], lay["row_slot"]
    blk_of_pos2d = blk_of_pos.reshape(nslot, 128)

    in_maps = [dict() for _ in range(NC)]
    for r in range(NC):
        mrk = core_o == r
        iarr = np.full(total, -1, np.int16)
        sarr = np.full(total, -1.0, np.float32)
        iarr[pos[mrk]] = ival[sorder][mrk]
        sarr[pos[mrk]] = dloc[sorder][mrk].astype(np.float32)
        # forward-fill per-core interior slack (cnt_r < L) with the run's
        # first real index so those gathers hit nearby/cached table rows;
        # run-tail rounding pads stay -1 (the Q7 ucode strips trailing -1s)
        cnt_r = cnt[r].reshape(-1)
        has = cnt_r > 0
        firsts = np.zeros(len(L_flat), np.int16)
        firsts[has] = iarr[gp_flat[has]]
        padmask = off_of_pos >= cnt_r[ordr[prun]]
        if cfg["STRIP"]:
            padmask = padmask & covered
        iarr[padmask] = firsts[ordr[prun][padmask]]
        # [16, total/16] wrapped layout; replicated to 128 partitions on-device
        in_maps[r]["idx"] = np.ascontiguousarray(iarr.reshape(-1, 16).T)

        # SELT row (b, s): lanes owned by block b keep their dloc, others -1
        sv = sarr.reshape(nslot, 128)[row_slot]
        sv = np.where(blk_of_pos2d[row_slot] == row_blk[:, None], sv, -1.0)
        selt = np.full((SB, NBAT * 128), -1.0, np.float32)
        rr = np.arange(len(row_blk))
        selt[(rr % SB)[:, None],
             ((rr // SB) * 128)[:, None] + lanes[None, :]] = sv
        in_maps[r]["selt"] = np.ascontiguousarray(selt.astype(BF16_NP))

    rhs4 = np.zeros((SB, SB * 128), np.float32)
    for k in range(SB):
        rhs4[k, k * 128:(k + 1) * 128] = 1.0
    w1s = W1.astype(BF16_NP)
    w2s = W2.astype(BF16_NP)
    b1b = np.tile(b1, (128, 1)).astype(np.float32)
    b2b = np.tile(b2, (128, 1)).astype(np.float32)

    dinv_by_pos = np.zeros(NP, np.float32)
    dinv_by_pos[node_pos] = dinv[:N]
    for r in range(NC):
        sh = dinv_by_pos[r * SHARD:(r + 1) * SHARD]
        in_maps[r]["dinv"] = np.ascontiguousarray(sh.reshape(BPC, 128).T)
        in_maps[r]["xt"] = np.ascontiguousarray(xt[r * SHARD:(r + 1) * SHARD])
        in_maps[r]["w1s"] = w1s
        in_maps[r]["w2s"] = w2s
        in_maps[r]["b1b"] = b1b
        in_maps[r]["b2b"] = b2b
        in_maps[r]["idm"] = np.eye(128, dtype=BF16_NP)
        in_maps[r]["rhs4"] = rhs4.astype(BF16_NP)

    return in_maps, L, lay, node_pos


def build_nc(cfg, L, lay, debug=False, sim_single=False):
    NC, BPC, BPG, NG = cfg["NCORES"], cfg["BPC"], cfg["BPG"], cfg["NG"]
    SHARD = cfg["SHARD"]
    QB, QN, QSTART = cfg["QB"], cfg["QN"], cfg["QSTART"]
    GCAP, NQ = cfg["GCAP"], cfg["NQ"]

    nc = bacc.Bacc("TRN2", target_bir_lowering=False, debug=debug,
                   num_devices=1 if sim_single else NC,
                   num_swdge_queues=NQ)

    total_pos = lay["total_pos"]
    NBAT = lay["n_selt"] // SB
    # per-group SELT batch ranges (block selt rows are SB-aligned)
    bs0 = lay["blk_selt0"] + [lay["n_selt"]]
    grp_bat0 = [bs0[g * BPG] // SB for g in range(NG + 1)]
    max_grp_bat = max(grp_bat0[g + 1] - grp_bat0[g] for g in range(NG))

    t_xt = nc.dram_tensor("xt", [SHARD, DIN], BF16, kind="ExternalInput")
    t_w1 = nc.dram_tensor("w1s", [DIN, DH], BF16, kind="ExternalInput")
    t_w2 = nc.dram_tensor("w2s", [DH, DOUT], BF16, kind="ExternalInput")
    t_b1 = nc.dram_tensor("b1b", [128, DH], F32, kind="ExternalInput")
    t_b2 = nc.dram_tensor("b2b", [128, DOUT], F32, kind="ExternalInput")
    t_idm = nc.dram_tensor("idm", [128, 128], BF16, kind="ExternalInput")
    t_rhs4 = nc.dram_tensor("rhs4", [SB, SB * 128], BF16, kind="ExternalInput")
    t_dinv = nc.dram_tensor("dinv", [128, BPC], F32, kind="ExternalInput")
    t_idx = nc.dram_tensor("idx", [16, total_pos // 16], I16, kind="ExternalInput")
    t_selt = nc.dram_tensor("selt", [SB, NBAT * 128], BF16, kind="ExternalInput")
    t_out = nc.dram_tensor("out", [SHARD, DOUT], BF16, kind="ExternalOutput")

    max_grp_pos = max(
        int(lay["grp_base"][g + 1] - lay["grp_base"][g]) for g in range(NG)
    )

    with tile.TileContext(nc) as tc:
        with (
            tc.tile_pool(name="const", bufs=1) as constp,
            tc.tile_pool(name="dram", bufs=1, space="DRAM") as dramp,
            tc.tile_pool(name="msg", bufs=2) as msgp,
            tc.tile_pool(name="selg", bufs=2) as stagp,
            tc.tile_pool(name="ind", bufs=10) as indp,
            tc.tile_pool(name="selb", bufs=3, space="PSUM") as selbp,
            tc.tile_pool(name="aggps", bufs=2, space="PSUM") as aggpsp,
            tc.tile_pool(name="xfps", bufs=2, space="PSUM") as xfpsp,
            tc.tile_pool(name="post", bufs=4) as postp,
        ):
            nc.gpsimd.load_library(library_config.mlp)

            W1 = constp.tile([DIN, DH], BF16)
            nc.sync.dma_start(W1[:], t_w1[:, :])
            W2 = constp.tile([DH, DOUT], BF16)
            nc.sync.dma_start(W2[:], t_w2[:, :])
            B1 = constp.tile([128, DH], F32)
            nc.sync.dma_start(B1[:], t_b1[:, :])
            B2 = constp.tile([128, DOUT], F32)
            nc.sync.dma_start(B2[:], t_b2[:, :])
            ID = constp.tile([128, 128], BF16)
            nc.sync.dma_start(ID[:], t_idm[:, :])
            IOTA4I = constp.tile([128, SB * 128], mybir.dt.int32)
            nc.gpsimd.iota(IOTA4I[:], [[0, SB], [1, 128]], base=0,
                           channel_multiplier=0)
            IOTA4 = constp.tile([128, SB * 128], F32)
            nc.vector.tensor_copy(IOTA4[:], IOTA4I[:])
            RHS4 = constp.tile([SB, SB * 128], BF16)
            nc.sync.dma_start(RHS4[:], t_rhs4[:, :])
            DINV = constp.tile([128, BPC], F32)
            nc.sync.dma_start(DINV[:], t_dinv[:, :])
            IDX = constp.tile([128, total_pos // 16], I16)
            for k in range(8):   # replicate the 16-row idx stream on-device
                nc.sync.dma_start(IDX[k * 16:(k + 1) * 16, :], t_idx[:, :])
            # local shard tables kept in SBUF for the per-block self-loop
            # identity matmuls (dloc on partitions, block-major columns)
            XTKEEP = constp.tile([128, BPC * DIN], BF16)
            for b in range(BPC):
                nc.sync.dma_start(XTKEEP[:, b * DIN:(b + 1) * DIN],
                                  t_xt[b * 128:(b + 1) * 128, :])
            H1KEEP = constp.tile([128, BPC * DH], BF16)

            xt_mine = [dramp.tile([QN[q], DIN], BF16, name=f"xtmine{q}")
                       for q in range(4)]
            xt_tab = [dramp.tile([QN[q] * NC, DIN], BF16, addr_space="Shared",
                                 name=f"xttab{q}") for q in range(4)]
            h1_mine = [dramp.tile([QN[q], DH], BF16, name=f"h1mine{q}")
                       for q in range(4)]
            h1_tab = [dramp.tile([QN[q] * NC, DH], BF16, addr_space="Shared",
                                 name=f"h1tab{q}") for q in range(4)]

            # assemble the full source table on-device from the 1/8 shard
            # (collectives can't read IO tensors: bounce through a DRAM tile)
            for q in range(4):
                r0 = int(QSTART[q]) * 128
                r1 = int(QSTART[q + 1]) * 128
                nc.sync.dma_start(xt_mine[q][:, :], t_xt[r0:r1, :])
                if sim_single:
                    nc.sync.dma_start(xt_tab[q][:QN[q], :], xt_mine[q][:, :])
                else:
                    nc.gpsimd.collective_compute(
                        "AllGather",
                        mybir.AluOpType.bypass,
                        replica_groups=[list(range(NC))],
                        ins=[xt_mine[q].opt()],
                        outs=[xt_tab[q].opt()],
                    )

            gpos = lay["gpos"]
            grp_base = lay["grp_base"]
            blk_slots = lay["blk_slots"]
            blk_selt0 = lay["blk_selt0"]

            def do_layer(layer):
                tabs = xt_tab if layer == 0 else h1_tab
                for g in range(NG):
                    p0 = int(grp_base[g])
                    p1 = int(grp_base[g + 1])
                    if p1 == p0:
                        continue
                    msg = msgp.tile([128, max_grp_pos // 128, DH], BF16, tag="msg")
                    if layer == 0 and g < 2:
                        # first use of each msg pool buffer: clear so lanes the
                        # gather skips (trailing -1 idx) can't hold NaN bits
                        nc.vector.memset(msg[:], 0)
                    gb0 = grp_bat0[g]
                    ngb = grp_bat0[g + 1] - gb0
                    SELG = stagp.tile([SB, max_grp_bat * 128], BF16, tag="selg")
                    nc.sync.dma_start(SELG[:, :ngb * 128],
                                      t_selt[:, gb0 * 128:(gb0 + ngb) * 128])
                    gq = 0
                    for c in range(4):
                        nidx = int(lay["run_len"][g, c])
                        if nidx == 0:
                            continue
                        # real (non-pad) indices in the run: trailing -1s are
                        # stripped by the ucode, so pass the exact count
                        nreal = int(L[g * BPG:(g + 1) * BPG, c].sum())
                        rp0 = int(gpos[g * BPG, c])  # global pos of run start
                        for s0 in range(0, nidx, GCAP):
                            n = min(GCAP, nidx - s0)
                            a0 = rp0 - p0 + s0       # pos offset in group buf
                            nc.gpsimd.dma_gather(
                                out_ap=msg[:, a0 // 128: (a0 + n) // 128, :],
                                in_ap=tabs[c][:, :],
                                idxs_ap=IDX[:, (rp0 + s0) // 16: (rp0 + s0 + n) // 16],
                                num_idxs=n,
                                num_idxs_reg=(max(0, min(n, nreal - s0))
                                              if cfg["STRIP"] else n),
                                elem_size=DH,
                                queue_num=gq % NQ,
                            )
                            gq += 1
                    for b in range(g * BPG, (g + 1) * BPG):
                        slots = blk_slots[b]
                        nbat = (len(slots) + SB - 1) // SB
                        inds = []
                        for t in range(nbat):
                            bt = blk_selt0[b] // SB + t - gb0  # batch in group
                            selb = selbp.tile([128, SB * 128], F32, tag="selb")
                            nc.tensor.matmul(
                                selb[:],
                                lhsT=SELG[:, bt * 128:(bt + 1) * 128],
                                rhs=RHS4[:],
                                start=True, stop=True,
                            )
                            ind = indp.tile([128, SB * 128], BF16, tag="ind")
                            nc.vector.tensor_tensor(
                                ind[:], selb[:], IOTA4[:],
                                mybir.AluOpType.is_equal,
                            )
                            inds.append(ind)
                        psA = aggpsp.tile([DH, 128], F32, tag="aggps")
                        # self-loop: prescaled local row d contributes to col d
                        keep = XTKEEP if layer == 0 else H1KEEP
                        dk = DIN if layer == 0 else DH
                        nc.tensor.matmul(
                            psA[:], lhsT=keep[:, b * dk:(b + 1) * dk], rhs=ID[:],
                            start=True, stop=(len(slots) == 0),
                        )
                        for k, s in enumerate(slots):
                            nc.tensor.matmul(
                                psA[:],
                                lhsT=msg[:, s - p0 // 128, :],
                                rhs=inds[k // SB][:, (k % SB) * 128:(k % SB + 1) * 128],
                                start=False, stop=(k == len(slots) - 1),
                            )
                        aggs = postp.tile([DH, 128], BF16, tag="aggs")
                        nc.vector.tensor_copy(aggs[:], psA[:])
                        dcol = DINV[:, b: b + 1]
                        if layer == 0:
                            psH = xfpsp.tile([128, DH], F32, tag="xfps")
                            nc.tensor.matmul(psH[:], lhsT=aggs[:], rhs=W1[:],
                                             start=True, stop=True)
                            tA = postp.tile([128, DH], F32, tag="tA")
                            nc.any.tensor_scalar(tA[:], psH[:], dcol, None,
                                                 mybir.AluOpType.mult)
                            tB = postp.tile([128, DH], F32, tag="tB")
                            nc.any.tensor_tensor(tB[:], tA[:], B1[:],
                                                 mybir.AluOpType.add)
                            # relu(x)*d == relu(x*d) for d>=0: fold the layer-2
                            # dinv[src] table prescale into the activation;
                            # write straight into the kept table for the
                            # layer-2 self-loop matmul
                            h1s = H1KEEP[:, b * DH:(b + 1) * DH]
                            nc.scalar.activation(
                                h1s, tB[:],
                                mybir.ActivationFunctionType.Relu, scale=dcol,
                            )
                            q = int(_block_quarter(cfg, b))
                            r0 = (b - int(QSTART[q])) * 128
                            nc.sync.dma_start(h1_mine[q][r0:r0 + 128, :], h1s)
                            if b == int(QSTART[q + 1]) - 1:
                                if sim_single:
                                    nc.sync.dma_start(
                                        h1_tab[q][:QN[q], :], h1_mine[q][:, :])
                                else:
                                    nc.gpsimd.collective_compute(
                                        "AllGather",
                                        mybir.AluOpType.bypass,
                                        replica_groups=[list(range(NC))],
                                        ins=[h1_mine[q].opt()],
                                        outs=[h1_tab[q].opt()],
                                    )
                        else:
                            psO = xfpsp.tile([128, DOUT], F32, tag="xfps")
                            nc.tensor.matmul(psO[:], lhsT=aggs[:], rhs=W2[:],
                                             start=True, stop=True)
                            tA = postp.tile([128, DOUT], F32, tag="tA")
                            nc.any.tensor_scalar(tA[:], psO[:], dcol, None,
                                                 mybir.AluOpType.mult)
                            ot = postp.tile([128, DOUT], BF16, tag="ot")
                            nc.any.tensor_tensor(ot[:], tA[:], B2[:],
                                                 mybir.AluOpType.add)
                            nc.sync.dma_start(t_out[b * 128:(b + 1) * 128, :], ot[:])

            do_layer(0)
            do_layer(1)

    nc.compile()
    return nc


def kernel(x, edge_index, W1, b1, W2, b2):
    cfg = make_cfg(100000, 1600000)
    in_maps, L, lay, node_pos = preprocess(cfg, x, edge_index, W1, b1, W2, b2)
    nc = build_nc(cfg, L, lay, debug=False)
    from concourse import bass_utils
    res = bass_utils.run_bass_kernel_spmd(
        nc, in_maps, core_ids=list(range(cfg["NCORES"]))
    )
    out = np.concatenate([res.results[r]["out"] for r in range(cfg["NCORES"])],
                         axis=0)
    return np.ascontiguousarray(out[node_pos]).astype(np.float32)


# revision 35
# speedup vs baseline: 1.3456x; 1.0202x over previous
"""GCN encoder (2-layer PyG-style GCNConv) as a Bass/Tile kernel on 8 trn2 NeuronCores.

Strategy (graph/data parallel, per sharding hint):
  - Nodes are partitioned across the 8 cores (12544 padded positions each, in
    degree-balanced serpentine order); each core aggregates all edges whose
    destination lands in its shard.
  - The feature table is SHARDED: each core uploads only its own 12544-row
    slice (bf16, dinv[src]-prescaled); the full table is assembled on-device
    with 4 quarter-chunk AllGathers (same flow as the layer-1 hidden table).
  - Both layers use identical position-based indexing, so the gather-index
    stream and the dst-selector stream are shared between layers and stay
    SBUF-resident.
  - Aggregation per 128-node dst block: for each 128-edge tile, gather the
    source rows with dma_gather (bf16), then accumulate indicator-weighted
    messages into PSUM via PE matmuls: aggT[f,d] += msg[e,f]^T @ ind[e,d].
  - Indicators are built 4 slots at a time: a K=4 PE matmul broadcasts the
    4 slots' dst-selector rows into a [128, 512] PSUM tile, and ONE wide DVE
    is_equal against a tiled iota constant produces all 4 indicator blocks
    (4x fewer DVE instructions than per-slot tensor_scalar builds).
  - GCN normalization deg^-1/2 A_hat deg^-1/2: table rows pre-scaled by
    dinv[src]; dinv[dst] applied per block after the dense transform.
"""

import sys

sys.path.insert(0, "/opt/trn_rl_repo")

import numpy as np

import concourse.bass as bass
import concourse.bacc as bacc
import concourse.mybir as mybir
from concourse import tile, library_config

BF16 = mybir.dt.bfloat16
F32 = mybir.dt.float32
I16 = mybir.dt.int16
BF16_NP = mybir.dt.np(BF16)

DIN, DH, DOUT = 128, 128, 64
SB = 4  # indicator slots built per SELB matmul / wide is_equal


def make_cfg(n_nodes, n_edges, n_cores=8, bpc=98, bpg=7, q_blocks=(25, 25, 24, 24),
             gcap=896, n_queues=2, strip=True):
    cfg = {}
    # strip=True: run-tail pads are -1 so the Q7 ucode skips them (HW).
    # CoreSim NaN-poisons skipped lanes, so sim verification uses strip=False
    # (forward-filled pads; identical layout and instruction stream).
    cfg["STRIP"] = strip
    cfg["N"] = n_nodes
    cfg["E"] = n_edges
    cfg["GCAP"] = gcap          # max indices per dma_gather instruction
    cfg["NQ"] = n_queues        # SWDGE queues to spread gathers over
    cfg["NCORES"] = n_cores
    cfg["BPC"] = bpc                      # dst blocks (of 128 nodes) per core
    cfg["BPG"] = bpg                      # blocks per gather group
    assert bpc % bpg == 0
    cfg["NG"] = bpc // bpg                # gather groups per core
    cfg["SHARD"] = bpc * 128              # padded nodes per core
    cfg["NP"] = n_cores * cfg["SHARD"]    # padded total nodes
    assert cfg["NP"] >= n_nodes
    assert sum(q_blocks) == bpc and len(q_blocks) == 4
    cfg["QB"] = list(q_blocks)            # blocks per quarter (collective chunks)
    cfg["QSTART"] = np.concatenate([[0], np.cumsum(q_blocks)])  # block ids
    cfg["QN"] = [q * 128 for q in q_blocks]   # nodes per quarter per rank
    for q in q_blocks:
        assert q * 128 * n_cores <= 32767
    return cfg


def _block_quarter(cfg, blk):
    """quarter id for a block index (vectorized)."""
    return np.searchsorted(cfg["QSTART"][1:], blk, side="right")


def make_layout(cfg, L):
    """Static slot/position layout from the per-(block, chunk) length table
    L [BPC, 4] (max real count over cores, identical across cores).

    Runs are per (group, chunk) and quantized to 128 at RUN level only:
    blocks within a run abut directly, so 128-edge slots at block boundaries
    are shared by adjacent blocks (each processes the slot with a selector
    row that masks the other block's lanes)."""
    BPC, BPG, NG = cfg["BPC"], cfg["BPG"], cfg["NG"]
    gpos = np.zeros((BPC, 4), np.int64)      # global position base of (b, c)
    run_len = np.zeros((NG, 4), np.int64)    # padded positions per (g, c) run
    grp_base = np.zeros(NG + 1, np.int64)    # global position base of group g
    p = 0
    for g in range(NG):
        grp_base[g] = p
        for c in range(4):
            for b in range(g * BPG, (g + 1) * BPG):
                gpos[b, c] = p
                p += int(L[b, c])
            rl = p - int(gpos[g * BPG, c])
            rl_pad = (rl + 127) // 128 * 128
            run_len[g, c] = rl_pad
            p = int(gpos[g * BPG, c]) + rl_pad
    grp_base[NG] = p

    # per-block slot lists (incl. shared boundary slots) + 4-aligned SELT rows
    blk_slots = []
    blk_selt0 = []
    row_blk, row_slot = [], []
    r = 0
    for b in range(BPC):
        slots = []
        for c in range(4):
            s0 = int(gpos[b, c])
            n = int(L[b, c])
            if n == 0:
                continue
            for s in range(s0 // 128, (s0 + n - 1) // 128 + 1):
                slots.append(s)
        blk_slots.append(slots)
        blk_selt0.append(r)
        for s in slots:
            row_blk.append(b)
            row_slot.append(s)
            r += 1
        while r % SB:
            row_blk.append(-1)
            row_slot.append(0)
            r += 1
    return {
        "gpos": gpos,
        "run_len": run_len,
        "grp_base": grp_base,
        "total_pos": p,
        "total_slots": p // 128,
        "blk_slots": blk_slots,
        "blk_selt0": blk_selt0,
        "row_blk": np.asarray(row_blk, np.int64),
        "row_slot": np.asarray(row_slot, np.int64),
        "n_selt": r,
    }


def preprocess(cfg, x, edge_index, W1, b1, W2, b2):
    """Host-side sharding: bucket/sort edges, build the shared per-core gather
    index + dst-selector streams, degree normalization, bf16 tables."""
    N, NP, NC = cfg["N"], cfg["NP"], cfg["NCORES"]
    SHARD, BPC, BPG = cfg["SHARD"], cfg["BPC"], cfg["BPG"]

    x = np.asarray(x, np.float32)
    edge_index = np.asarray(edge_index)
    W1 = np.asarray(W1, np.float32)
    b1 = np.asarray(b1, np.float32)
    W2 = np.asarray(W2, np.float32)
    b2 = np.asarray(b2, np.float32)

    # self-loops are NOT in the gather stream: they are added per block on
    # device via an identity matmul from the locally-kept (prescaled) tables
    src = edge_index[0].astype(np.int64)
    dst = edge_index[1].astype(np.int64)

    deg = np.bincount(dst, minlength=NP).astype(np.float32)
    deg[:N] += 1.0                      # A_hat = A + I
    dinv = np.zeros(NP, np.float32)
    nz = deg > 0
    dinv[nz] = 1.0 / np.sqrt(deg[nz])

    # degree-balanced node -> (core, block, slot) packing: serpentine deal of
    # nodes sorted by in-degree so every 128-node block has ~equal edge count
    NB = NP // 128
    order = np.argsort(-deg[:N], kind="stable")
    ids = np.concatenate([order, np.full(NP - N, -1, np.int64)])
    rounds = ids.reshape(128, NB).copy()
    rounds[1::2] = rounds[1::2, ::-1]
    posmat = (np.arange(NB)[None, :] * 128 + np.arange(128)[:, None])
    node_pos = np.zeros(N, np.int64)
    m = rounds >= 0
    node_pos[rounds[m]] = posmat[m]

    # position-ordered dinv[src]-prescaled bf16 table (sharded per core)
    xs = np.zeros((NP, DIN), np.float32)
    xs[node_pos] = x * dinv[:N, None]
    xt = xs.astype(BF16_NP)

    p_dst = node_pos[dst]
    core = (p_dst // SHARD).astype(np.int32)
    blk = ((p_dst % SHARD) // 128).astype(np.int32)   # block within core
    dloc = (p_dst % 128).astype(np.int32)
    grp = blk // BPG

    # shared (both layers) position-quarter chunking of the source index
    p_src = node_pos[src]
    s_rank = p_src // SHARD
    s_loc = p_src % SHARD
    s_blk = (s_loc // 128).astype(np.int32)
    cl = _block_quarter(cfg, s_blk).astype(np.int32)
    qn = np.asarray(cfg["QN"], np.int64)
    qstart_nodes = cfg["QSTART"][:4] * 128
    ival = (s_rank * qn[cl] + (s_loc - qstart_nodes[cl])).astype(np.int16)

    # per-core per-(block, chunk) counts -> shared max-count length table
    key = (core.astype(np.int64) * BPC + blk) * 4 + cl
    cnt = np.bincount(key, minlength=NC * BPC * 4).reshape(NC, BPC, 4)
    L = cnt.max(axis=0)
    lay = make_layout(cfg, L)

    # stable sort: (core, group, chunk, block, src)
    sorder = np.lexsort((src, blk, cl, grp, core))
    ekey = key[sorder]
    change = np.r_[True, ekey[1:] != ekey[:-1]]
    starts = np.flatnonzero(change)
    runid = np.cumsum(change) - 1
    within = np.arange(len(ekey)) - starts[runid]
    gp = lay["gpos"]
    pos = gp[blk[sorder], cl[sorder]] + within
    core_o = core[sorder]

    total = lay["total_pos"]
    nslot = lay["total_slots"]
    gp_flat = gp.reshape(-1)
    L_flat = L.reshape(-1)
    NBAT = lay["n_selt"] // SB
    lanes = np.arange(128)

    # map every position to its owning (block, chunk) run; positions beyond
    # the runs' real extents (run-tail 128-rounding) have owner -1
    ordr = np.argsort(gp_flat, kind="stable")
    gp_sorted = gp_flat[ordr]
    L_sorted = L_flat[ordr]
    allpos = np.arange(total)
    prun = np.searchsorted(gp_sorted, allpos, side="right") - 1
    off_of_pos = allpos - gp_sorted[prun]
    covered = off_of_pos < L_sorted[prun]
    blk_of_pos = np.where(covered, ordr[prun] // 4, -1)

    row_blk, row_slot = lay["row_blk"], lay["row_slot"]
    blk_of_pos2d = blk_of_pos.reshape(nslot, 128)

    in_maps = [dict() for _ in range(NC)]
    for r in range(NC):
        mrk = core_o == r
        iarr = np.full(total, -1, np.int16)
        sarr = np.full(total, -1.0, np.float32)
        iarr[pos[mrk]] = ival[sorder][mrk]
        sarr[pos[mrk]] = dloc[sorder][mrk].astype(np.float32)
        # forward-fill per-core interior slack (cnt_r < L) with the run's
        # first real index so those gathers hit nearby/cached table rows;
        # run-tail rounding pads stay -1 (the Q7 ucode strips trailing -1s)
        cnt_r = cnt[r].reshape(-1)
        has = cnt_r > 0
        firsts = np.zeros(len(L_flat), np.int16)
        firsts[has] = iarr[gp_flat[has]]
        padmask = off_of_pos >= cnt_r[ordr[prun]]
        if cfg["STRIP"]:
            padmask = padmask & covered
        iarr[padmask] = firsts[ordr[prun][padmask]]
        # [16, total/16] wrapped layout; replicated to 128 partitions on-device
        in_maps[r]["idx"] = np.ascontiguousarray(iarr.reshape(-1, 16).T)

        # SELT row (b, s): lanes owned by block b keep their dloc, others -1
        sv = sarr.reshape(nslot, 128)[row_slot]
        sv = np.where(blk_of_pos2d[row_slot] == row_blk[:, None], sv, -1.0)
        selt = np.full((SB, NBAT * 128), -1.0, np.float32)
        rr = np.arange(len(row_blk))
        selt[(rr % SB)[:, None],
             ((rr // SB) * 128)[:, None] + lanes[None, :]] = sv
        in_maps[r]["selt"] = np.ascontiguousarray(selt.astype(BF16_NP))

    rhs4 = np.zeros((SB, SB * 128), np.float32)
    for k in range(SB):
        rhs4[k, k * 128:(k + 1) * 128] = 1.0
    w1s = W1.astype(BF16_NP)
    w2s = W2.astype(BF16_NP)
    b1b = np.tile(b1, (128, 1)).astype(np.float32)
    b2b = np.tile(b2, (128, 1)).astype(np.float32)

    dinv_by_pos = np.zeros(NP, np.float32)
    dinv_by_pos[node_pos] = dinv[:N]
    for r in range(NC):
        sh = dinv_by_pos[r * SHARD:(r + 1) * SHARD]
        in_maps[r]["dinv"] = np.ascontiguousarray(sh.reshape(BPC, 128).T)
        in_maps[r]["xt"] = np.ascontiguousarray(xt[r * SHARD:(r + 1) * SHARD])
        in_maps[r]["w1s"] = w1s
        in_maps[r]["w2s"] = w2s
        in_maps[r]["b1b"] = b1b
        in_maps[r]["b2b"] = b2b
        in_maps[r]["idm"] = np.eye(128, dtype=BF16_NP)
        in_maps[r]["rhs4"] = rhs4.astype(BF16_NP)

    return in_maps, L, lay, node_pos


def build_nc(cfg, L, lay, debug=False, sim_single=False):
    NC, BPC, BPG, NG = cfg["NCORES"], cfg["BPC"], cfg["BPG"], cfg["NG"]
    SHARD = cfg["SHARD"]
    QB, QN, QSTART = cfg["QB"], cfg["QN"], cfg["QSTART"]
    GCAP, NQ = cfg["GCAP"], cfg["NQ"]

    nc = bacc.Bacc("TRN2", target_bir_lowering=False, debug=debug,
                   num_devices=1 if sim_single else NC,
                   num_swdge_queues=NQ)

    total_pos = lay["total_pos"]
    NBAT = lay["n_selt"] // SB
    # per-group SELT batch ranges (block selt rows are SB-aligned)
    bs0 = lay["blk_selt0"] + [lay["n_selt"]]
    grp_bat0 = [bs0[g * BPG] // SB for g in range(NG + 1)]
    max_grp_bat = max(grp_bat0[g + 1] - grp_bat0[g] for g in range(NG))

    t_xt = nc.dram_tensor("xt", [SHARD, DIN], BF16, kind="ExternalInput")
    t_w1 = nc.dram_tensor("w1s", [DIN, DH], BF16, kind="ExternalInput")
    t_w2 = nc.dram_tensor("w2s", [DH, DOUT], BF16, kind="ExternalInput")
    t_b1 = nc.dram_tensor("b1b", [128, DH], F32, kind="ExternalInput")
    t_b2 = nc.dram_tensor("b2b", [128, DOUT], F32, kind="ExternalInput")
    t_idm = nc.dram_tensor("idm", [128, 128], BF16, kind="ExternalInput")
    t_rhs4 = nc.dram_tensor("rhs4", [SB, SB * 128], BF16, kind="ExternalInput")
    t_dinv = nc.dram_tensor("dinv", [128, BPC], F32, kind="ExternalInput")
    t_idx = nc.dram_tensor("idx", [16, total_pos // 16], I16, kind="ExternalInput")
    t_selt = nc.dram_tensor("selt", [SB, NBAT * 128], BF16, kind="ExternalInput")
    t_out = nc.dram_tensor("out", [SHARD, DOUT], BF16, kind="ExternalOutput")

    max_grp_pos = max(
        int(lay["grp_base"][g + 1] - lay["grp_base"][g]) for g in range(NG)
    )

    with tile.TileContext(nc) as tc:
        with (
            tc.tile_pool(name="const", bufs=1) as constp,
            tc.tile_pool(name="dram", bufs=1, space="DRAM") as dramp,
            tc.tile_pool(name="msg", bufs=2) as msgp,
            tc.tile_pool(name="selg", bufs=2) as stagp,
            tc.tile_pool(name="ind", bufs=10) as indp,
            tc.tile_pool(name="selb", bufs=3, space="PSUM") as selbp,
            tc.tile_pool(name="aggps", bufs=2, space="PSUM") as aggpsp,
            tc.tile_pool(name="xfps", bufs=2, space="PSUM") as xfpsp,
            tc.tile_pool(name="post", bufs=4) as postp,
        ):
            nc.gpsimd.load_library(library_config.mlp)

            xt_mine = [dramp.tile([QN[q], DIN], BF16, name=f"xtmine{q}")
                       for q in range(4)]
            xt_tab = [dramp.tile([QN[q] * NC, DIN], BF16, addr_space="Shared",
                                 name=f"xttab{q}") for q in range(4)]
            h1_mine = [dramp.tile([QN[q], DH], BF16, name=f"h1mine{q}")
                       for q in range(4)]
            h1_tab = [dramp.tile([QN[q] * NC, DH], BF16, addr_space="Shared",
                                 name=f"h1tab{q}") for q in range(4)]

            # assemble the full source table on-device from the 1/8 shard
            # FIRST (collectives can't read IO tensors: bounce through a DRAM
            # tile) — the first gather waits on AllGather q0, so these lead
            # the sync DMA queue while the big constant loads go via scalar
            for q in range(4):
                r0 = int(QSTART[q]) * 128
                r1 = int(QSTART[q + 1]) * 128
                nc.sync.dma_start(xt_mine[q][:, :], t_xt[r0:r1, :])
                if sim_single:
                    nc.sync.dma_start(xt_tab[q][:QN[q], :], xt_mine[q][:, :])
                else:
                    nc.gpsimd.collective_compute(
                        "AllGather",
                        mybir.AluOpType.bypass,
                        replica_groups=[list(range(NC))],
                        ins=[xt_mine[q].opt()],
                        outs=[xt_tab[q].opt()],
                    )

            IDX = constp.tile([128, total_pos // 16], I16)
            for k in range(8):   # replicate the 16-row idx stream on-device
                nc.scalar.dma_start(IDX[k * 16:(k + 1) * 16, :], t_idx[:, :])
            W1 = constp.tile([DIN, DH], BF16)
            nc.scalar.dma_start(W1[:], t_w1[:, :])
            W2 = constp.tile([DH, DOUT], BF16)
            nc.scalar.dma_start(W2[:], t_w2[:, :])
            B1 = constp.tile([128, DH], F32)
            nc.scalar.dma_start(B1[:], t_b1[:, :])
            B2 = constp.tile([128, DOUT], F32)
            nc.scalar.dma_start(B2[:], t_b2[:, :])
            ID = constp.tile([128, 128], BF16)
            nc.scalar.dma_start(ID[:], t_idm[:, :])
            IOTA4I = constp.tile([128, SB * 128], mybir.dt.int32)
            nc.gpsimd.iota(IOTA4I[:], [[0, SB], [1, 128]], base=0,
                           channel_multiplier=0)
            IOTA4 = constp.tile([128, SB * 128], F32)
            nc.vector.tensor_copy(IOTA4[:], IOTA4I[:])
            RHS4 = constp.tile([SB, SB * 128], BF16)
            nc.scalar.dma_start(RHS4[:], t_rhs4[:, :])
            DINV = constp.tile([128, BPC], F32)
            nc.scalar.dma_start(DINV[:], t_dinv[:, :])
            # local shard tables kept in SBUF for the per-block self-loop
            # identity matmuls (dloc on partitions, block-major columns)
            XTKEEP = constp.tile([128, BPC * DIN], BF16)
            for b in range(BPC):
                nc.scalar.dma_start(XTKEEP[:, b * DIN:(b + 1) * DIN],
                                    t_xt[b * 128:(b + 1) * 128, :])
            H1KEEP = constp.tile([128, BPC * DH], BF16)

            gpos = lay["gpos"]
            grp_base = lay["grp_base"]
            blk_slots = lay["blk_slots"]
            blk_selt0 = lay["blk_selt0"]

            def do_layer(layer):
                tabs = xt_tab if layer == 0 else h1_tab
                for g in range(NG):
                    p0 = int(grp_base[g])
                    p1 = int(grp_base[g + 1])
                    if p1 == p0:
                        continue
                    msg = msgp.tile([128, max_grp_pos // 128, DH], BF16, tag="msg")
                    if layer == 0 and g < 2:
                        # first use of each msg pool buffer: clear so lanes the
                        # gather skips (trailing -1 idx) can't hold NaN bits
                        nc.vector.memset(msg[:], 0)
                    gb0 = grp_bat0[g]
                    ngb = grp_bat0[g + 1] - gb0
                    SELG = stagp.tile([SB, max_grp_bat * 128], BF16, tag="selg")
                    nc.sync.dma_start(SELG[:, :ngb * 128],
                                      t_selt[:, gb0 * 128:(gb0 + ngb) * 128])
                    gq = 0
                    for c in range(4):
                        nidx = int(lay["run_len"][g, c])
                        if nidx == 0:
                            continue
                        # real (non-pad) indices in the run: trailing -1s are
                        # stripped by the ucode, so pass the exact count
                        nreal = int(L[g * BPG:(g + 1) * BPG, c].sum())
                        rp0 = int(gpos[g * BPG, c])  # global pos of run start
                        for s0 in range(0, nidx, GCAP):
                            n = min(GCAP, nidx - s0)
                            a0 = rp0 - p0 + s0       # pos offset in group buf
                            nc.gpsimd.dma_gather(
                                out_ap=msg[:, a0 // 128: (a0 + n) // 128, :],
                                in_ap=tabs[c][:, :],
                                idxs_ap=IDX[:, (rp0 + s0) // 16: (rp0 + s0 + n) // 16],
                                num_idxs=n,
                                num_idxs_reg=(max(0, min(n, nreal - s0))
                                              if cfg["STRIP"] else n),
                                elem_size=DH,
                                queue_num=gq % NQ,
                            )
                            gq += 1
                    for b in range(g * BPG, (g + 1) * BPG):
                        slots = blk_slots[b]
                        nbat = (len(slots) + SB - 1) // SB
                        inds = []
                        for t in range(nbat):
                            bt = blk_selt0[b] // SB + t - gb0  # batch in group
                            selb = selbp.tile([128, SB * 128], F32, tag="selb")
                            nc.tensor.matmul(
                                selb[:],
                                lhsT=SELG[:, bt * 128:(bt + 1) * 128],
                                rhs=RHS4[:],
                                start=True, stop=True,
                            )
                            ind = indp.tile([128, SB * 128], BF16, tag="ind")
                            nc.vector.tensor_tensor(
                                ind[:], selb[:], IOTA4[:],
                                mybir.AluOpType.is_equal,
                            )
                            inds.append(ind)
                        psA = aggpsp.tile([DH, 128], F32, tag="aggps")
                        # self-loop: prescaled local row d contributes to col d
                        keep = XTKEEP if layer == 0 else H1KEEP
                        dk = DIN if layer == 0 else DH
                        nc.tensor.matmul(
                            psA[:], lhsT=keep[:, b * dk:(b + 1) * dk], rhs=ID[:],
                            start=True, stop=(len(slots) == 0),
                        )
                        for k, s in enumerate(slots):
                            nc.tensor.matmul(
                                psA[:],
                                lhsT=msg[:, s - p0 // 128, :],
                                rhs=inds[k // SB][:, (k % SB) * 128:(k % SB + 1) * 128],
                                start=False, stop=(k == len(slots) - 1),
                            )
                        aggs = postp.tile([DH, 128], BF16, tag="aggs")
                        nc.vector.tensor_copy(aggs[:], psA[:])
                        dcol = DINV[:, b: b + 1]
                        if layer == 0:
                            psH = xfpsp.tile([128, DH], F32, tag="xfps")
                            nc.tensor.matmul(psH[:], lhsT=aggs[:], rhs=W1[:],
                                             start=True, stop=True)
                            tA = postp.tile([128, DH], F32, tag="tA")
                            nc.any.tensor_scalar(tA[:], psH[:], dcol, None,
                                                 mybir.AluOpType.mult)
                            tB = postp.tile([128, DH], F32, tag="tB")
                            nc.any.tensor_tensor(tB[:], tA[:], B1[:],
                                                 mybir.AluOpType.add)
                            # relu(x)*d == relu(x*d) for d>=0: fold the layer-2
                            # dinv[src] table prescale into the activation;
                            # write straight into the kept table for the
                            # layer-2 self-loop matmul
                            h1s = H1KEEP[:, b * DH:(b + 1) * DH]
                            nc.scalar.activation(
                                h1s, tB[:],
                                mybir.ActivationFunctionType.Relu, scale=dcol,
                            )
                            q = int(_block_quarter(cfg, b))
                            r0 = (b - int(QSTART[q])) * 128
                            nc.sync.dma_start(h1_mine[q][r0:r0 + 128, :], h1s)
                            if b == int(QSTART[q + 1]) - 1:
                                if sim_single:
                                    nc.sync.dma_start(
                                        h1_tab[q][:QN[q], :], h1_mine[q][:, :])
                                else:
                                    nc.gpsimd.collective_compute(
                                        "AllGather",
                                        mybir.AluOpType.bypass,
                                        replica_groups=[list(range(NC))],
                                        ins=[h1_mine[q].opt()],
                                        outs=[h1_tab[q].opt()],
                                    )
                        else:
                            psO = xfpsp.tile([128, DOUT], F32, tag="xfps")
                            nc.tensor.matmul(psO[:], lhsT=aggs[:], rhs=W2[:],
                                             start=True, stop=True)
                            tA = postp.tile([128, DOUT], F32, tag="tA")
                            nc.any.tensor_scalar(tA[:], psO[:], dcol, None,
                                                 mybir.AluOpType.mult)
                            ot = postp.tile([128, DOUT], BF16, tag="ot")
                            nc.any.tensor_tensor(ot[:], tA[:], B2[:],
                                                 mybir.AluOpType.add)
                            nc.sync.dma_start(t_out[b * 128:(b + 1) * 128, :], ot[:])

            do_layer(0)
            do_layer(1)

    nc.compile()
    return nc


def kernel(x, edge_index, W1, b1, W2, b2):
    cfg = make_cfg(100000, 1600000)
    in_maps, L, lay, node_pos = preprocess(cfg, x, edge_index, W1, b1, W2, b2)
    nc = build_nc(cfg, L, lay, debug=False)
    from concourse import bass_utils
    res = bass_utils.run_bass_kernel_spmd(
        nc, in_maps, core_ids=list(range(cfg["NCORES"]))
    )
    out = np.concatenate([res.results[r]["out"] for r in range(cfg["NCORES"])],
                         axis=0)
    return np.ascontiguousarray(out[node_pos]).astype(np.float32)
